# revision 7
# baseline (speedup 1.0000x reference)
"""AFM (Attentional Factorization Machine) Trainium2 kernel, 8-core data-parallel.

Reference computation (per batch row b):
  emb        = embed_table[sparse_feat[b]]                      [26, 64]
  linear_out = sum_f lin_table[sparse_feat[b,f]] + dense.w_d + bias
  att_x[p]   = emb[i_p] * emb[j_p]          (325 pairs i<j)     [325, 64]
  h          = relu(att_x @ W1 + b1);  att_w = h @ w2           [325]
  score      = softmax(att_w);  out = linear_out + (sum_p score_p att_x[p]) . p

Strategy: shard batch (4096 -> 512/core).  Host prep does the embedding
gather AND the transpose, shipping a ready-to-use embT per core:
  embT [128 = d + 64*g, 26*256], g = b//256 (two batch-groups packed on
  partitions), col = f*256 + (b%256).  Device just DMAs it (1.7MB,
  split into 7 four-field parts so chunk compute starts as soon as the
  first fields land).
Per core:
  - pairs processed j-major: chunk j covers pairs (i<j) -> att_x chunk via DVE
    tensor_tensor with step-0 broadcast of emb_j over the i-run, bf16.
  - mm1: lhsT = blockdiag(W1,W1) [128,128], stream attx -> h psum [128,1024].
  - relu+bias via ACT (one op per 1024 cols) -> relu_h bf16.
  - att_w: per 128-col tile, matmul(lhsT=relu_h_tile, rhs=[[w2,0],[0,w2]]) ->
    psum [128 b, 2];  q likewise from attx with [[p],[p]].  b-on-partitions.
  - softmax over pairs without max subtraction (logits are tiny); exp+sum
    fused on ACT via accum_out; out = linear + (sum ew*q)/Z.
"""

import os
import sys

sys.path.insert(0, "/opt/trn_rl_repo")

import numpy as np
import ml_dtypes

from concourse import bass, library_config, mybir, tile
from concourse import bass_utils as _bu
from concourse.bass_utils import run_bass_kernel_spmd
from concourse.masks import make_identity

BF16 = mybir.dt.bfloat16
FP8 = mybir.dt.float8e4
F32 = mybir.dt.float32
I32 = mybir.dt.int32

EMB_SCALE = 1.0
UNSCALE = 1.0 / (EMB_SCALE * EMB_SCALE)

V = 100000
F = 26
D = 64
BC = 512  # batch per core
NCORES = 8
NPAIR = F * (F - 1) // 2  # 325
NPART = (F + 3) // 4  # 7 four-field parts


def _split_excess_waits(nc, cap=1):
    """This walrus build gives most instruction structs a single embedded
    sync-wait slot; Tile can emit more.  Hoist extra waits onto injected
    same-engine NoOps right before the instruction (engine FIFO makes the
    combined wait-set semantics identical)."""
    skip = {"EventSemaphore", "AllEngineBarrier", "UnconditionalBranch",
            "CompareAndBranch", "BranchHint"}
    for fn in nc.m.functions:
        for blk in fn.blocks:
            new_insts = []
            for inst in blk.instructions:
                if inst.opcode == "Ldweights" and inst.tile_position == (0, 0):
                    # a (0,0)/full tile_position is a no-op but trips walrus's
                    # "not compatible with LDW optimization" check
                    inst.tile_position = None
                    inst.tile_size = None
                si = inst.sync_info
                if (
                    si is not None
                    and si.on_wait
                    and len(si.on_wait) > cap
                    and inst.opcode not in skip
                ):
                    waits = list(si.on_wait)
                    for w in waits[:-cap]:
                        nop = mybir.InstNoOp(
                            name=nc.get_next_instruction_name(),
                            engine=inst.engine,
                            bass_nofuse=True,
                            sync_info=mybir.SyncInfo(on_wait=[w], on_update=[]),
                        )
                        new_insts.append(nop)
                    si.on_wait = waits[-cap:]
                new_insts.append(inst)
            blk.instructions = new_insts
    return nc


def build_nc(split_waits=True):
    nc = bass.Bass()

    embt_d = nc.declare_dram_parameter("embt", [128, F * 256], BF16, isOutput=False)
    lin_d = nc.declare_dram_parameter("lin", [128, 4 * F], F32, isOutput=False)
    dense = nc.declare_dram_parameter("dense", [128, 70], F32, isOutput=False)
    w1blk = nc.declare_dram_parameter("w1blk", [128, 128], BF16, isOutput=False)
    b1c = nc.declare_dram_parameter("b1c", [128, 1], F32, isOutput=False)
    w2blk = nc.declare_dram_parameter("w2blk", [128, 2], BF16, isOutput=False)
    pblk = nc.declare_dram_parameter("pblk", [128, 2], BF16, isOutput=False)
    out_d = nc.declare_dram_parameter("out", [BC], F32, isOutput=True)

    NTILE = 2 * NPAIR  # 650 column-tiles of 128 (pair x bhalf)
    NWCOL = 2 * NTILE  # 1300 psum output cols (x2 groups)

    with tile.TileContext(nc) as tc:
        with (
            tc.tile_pool(name="const", bufs=1) as cpool,
            tc.tile_pool(name="ax", bufs=3) as axpool,
            tc.tile_pool(name="rh", bufs=4) as rhpool,
            tc.tile_pool(name="ps", bufs=2, space="PSUM") as pspool,
        ):
            # ---- persistent sbuf tensors ----
            embt_p = [
                cpool.tile(
                    [128, min(4, F - 4 * pp) * 256], BF16,
                    tag=f"embt{pp}", name=f"embt{pp}",
                )
                for pp in range(NPART)
            ]
            lin_sb = cpool.tile([128, 4 * F], F32, tag="lin")
            densewd_sb = cpool.tile([128, 70], F32, tag="dense")
            dense_sb = densewd_sb[:, 0:56]
            wd_sb = densewd_sb[:, 56:70]
            w1_sb = cpool.tile([128, 128], BF16, tag="w1")
            b1_sb = cpool.tile([128, 1], F32, tag="b1")
            w2_sb = cpool.tile([128, 2], BF16, tag="w2")
            p_sb = cpool.tile([128, 2], BF16, tag="p")
            ident = cpool.tile([128, 128], BF16, tag="ident")
            attw_sb = cpool.tile([128, NWCOL], F32, tag="attw")
            q_sb = cpool.tile([128, NWCOL], F32, tag="q")
            ew_sb = cpool.tile([128, NWCOL], F32, tag="ew")
            prod_sb = cpool.tile([128, NWCOL], F32, tag="prod")
            lsum_sb = cpool.tile([128, 4], F32, tag="lsum")
            dtmp_sb = cpool.tile([128, 4 * 14], F32, tag="dtmp")
            dsum_sb = cpool.tile([128, 4], F32, tag="dsum")
            lino_sb = cpool.tile([128, 4], F32, tag="lino")
            z_sb = cpool.tile([128, 4], F32, tag="z")
            zi_sb = cpool.tile([128, 4], F32, tag="zi")
            s4_sb = cpool.tile([128, 4], F32, tag="s4")
            fm_sb = cpool.tile([128, 4], F32, tag="fm")
            out_sb = cpool.tile([128, 4], F32, tag="outsb")

            # ---- load parameters ----
            # Small params first (their triggers are fast and W1/b1 gate the
            # first matmul/relu); embt triggers spread across the other
            # engine sequencers so the ~600ns DIRECT2D issues parallelize.
            nc.sync.dma_start(out=w1_sb[:], in_=w1blk[:, :])
            nc.sync.dma_start(out=b1_sb[:], in_=b1c[:, :])
            nc.sync.dma_start(out=w2_sb[:], in_=w2blk[:, :])
            nc.sync.dma_start(out=p_sb[:], in_=pblk[:, :])
            trig = [nc.scalar, nc.gpsimd, nc.sync]
            for pp in range(NPART):
                nf = min(4, F - 4 * pp)
                trig[pp % 3].dma_start(
                    out=embt_p[pp][:],
                    in_=embt_d[:, pp * 1024 : pp * 1024 + nf * 256],
                )
            nc.sync.dma_start(out=lin_sb[:], in_=lin_d[:, :])
            nc.sync.dma_start(out=densewd_sb[:], in_=dense[:, :])
            make_identity(nc, ident[:])

            # ---- chunk j: pairs (i, j) for i < j ----
            state = {
                "tdone": 0, "copied": 0, "ps_w": None, "ps_q": None,
                "gidx": 0, "axidx": 0, "pend": [],
            }

            def emit_mm2s(rh, ax, n0, N):
                """Second-stage matmuls for one 1024-col group (emitted one
                group late so PE never waits on the relu of the group it is
                currently streaming)."""
                for t0 in range(0, N, 128):
                    u = state["tdone"] % 256
                    if u == 0:
                        state["ps_w"] = pspool.tile(
                            [128, 512], F32, tag="w", name=f"psw{state['tdone']}"
                        )
                        state["ps_q"] = pspool.tile(
                            [128, 512], F32, tag="q", name=f"psq{state['tdone']}"
                        )
                    # q first: it depends only on ax, not on the relu output
                    nc.tensor.matmul(
                        out=state["ps_q"][:, 2 * u : 2 * u + 2],
                        lhsT=ax[:, n0 + t0 : n0 + t0 + 128],
                        rhs=p_sb[:],
                        start=True,
                        stop=True,
                    )
                    nc.tensor.matmul(
                        out=state["ps_w"][:, 2 * u : 2 * u + 2],
                        lhsT=rh[:, t0 : t0 + 128],
                        rhs=w2_sb[:],
                        start=True,
                        stop=True,
                    )
                    state["tdone"] += 1
                    if state["tdone"] % 256 == 0 or state["tdone"] == NTILE:
                        w2c = 2 * ((state["tdone"] - 1) % 256 + 1)
                        c0 = state["copied"]
                        nc.vector.tensor_scalar(
                            out=attw_sb[:, c0 : c0 + w2c],
                            in0=state["ps_w"][:, :w2c],
                            scalar1=UNSCALE,
                            scalar2=None,
                            op0=mybir.AluOpType.mult,
                        )
                        nc.vector.tensor_scalar(
                            out=q_sb[:, c0 : c0 + w2c],
                            in0=state["ps_q"][:, :w2c],
                            scalar1=UNSCALE,
                            scalar2=None,
                            op0=mybir.AluOpType.mult,
                        )
                        state["copied"] += w2c

            def emit_chunk(j):
                jpp, jo = j // 4, j % 4
                ax = axpool.tile([128, 25 * 256], BF16, tag="ax", name=f"ax{j}")
                # absorb WAR-on-slot (PE readers) into a 1-wait-capable copy
                nc.vector.tensor_copy(out=ax[:, 0:1], in_=ident[:, 0:1])
                in0 = embt_p[jpp][:, jo * 256 : (jo + 1) * 256].rearrange(
                    "p (one w) -> p one w", one=1
                )
                for ib in range(0, j, 4):
                    w = min(4, j - ib)
                    # route 1 in 3 ax products to the otherwise-idle gpsimd
                    axeng = nc.gpsimd if state["axidx"] % 3 == 2 else nc.vector
                    state["axidx"] += 1
                    axeng.tensor_tensor(
                        out=ax[:, ib * 256 : (ib + w) * 256].rearrange(
                            "p (l w) -> p l w", l=w
                        ),
                        in0=in0.to_broadcast([128, w, 256]),
                        in1=embt_p[ib // 4][:, : w * 256].rearrange(
                            "p (l w) -> p l w", l=w
                        ),
                        op=mybir.AluOpType.mult,
                    )
                ncols = j * 256
                for n0 in range(0, ncols, 1024):
                    N = min(1024, ncols - n0)
                    hp = pspool.tile([128, 1024], F32, tag="h", name=f"hp{j}_{n0}")
                    for m0 in range(0, N, 512):
                        M = min(512, N - m0)
                        nc.tensor.matmul(
                            out=hp[:, m0 : m0 + M],
                            lhsT=w1_sb[:],
                            rhs=ax[:, n0 + m0 : n0 + m0 + M],
                            start=True,
                            stop=True,
                        )
                    rh = rhpool.tile([128, 1024], BF16, tag="rh", name=f"rh{j}_{n0}")
                    # split relu 5:2 between ACT and DVE
                    if state["gidx"] % 7 < 5:
                        nc.scalar.activation(
                            out=rh[:, :N],
                            in_=hp[:, :N],
                            func=mybir.ActivationFunctionType.Relu,
                            bias=b1_sb[:],
                        )
                    else:
                        nc.vector.tensor_scalar(
                            out=rh[:, :N],
                            in0=hp[:, :N],
                            scalar1=b1_sb[:, 0:1],
                            scalar2=0.0,
                            op0=mybir.AluOpType.add,
                            op1=mybir.AluOpType.max,
                        )
                    state["gidx"] += 1
                    state["pend"].append((rh, ax, n0, N))
                    if len(state["pend"]) > 1:
                        emit_mm2s(*state["pend"].pop(0))

            for j in range(1, F):
                emit_chunk(j)
            while state["pend"]:
                emit_mm2s(*state["pend"].pop(0))

            # ---- linear tail ----
            nc.vector.reduce_sum(
                out=lsum_sb[:],
                in_=lin_sb[:].rearrange("p (blk f) -> p blk f", blk=4),
                axis=mybir.AxisListType.X,
            )
            for blk in range(4):
                nc.vector.tensor_tensor(
                    out=dtmp_sb[:, blk * 14 : (blk + 1) * 14],
                    in0=dense_sb[:, blk * 14 : (blk + 1) * 14],
                    in1=wd_sb[:, :],
                    op=mybir.AluOpType.mult,
                )
            nc.vector.reduce_sum(
                out=dsum_sb[:],
                in_=dtmp_sb[:].rearrange("p (blk w) -> p blk w", blk=4),
                axis=mybir.AxisListType.X,
            )
            nc.vector.tensor_add(out=lino_sb[:], in0=lsum_sb[:], in1=dsum_sb[:])

            # ---- softmax (no max subtraction; logits are tiny) ----
            attw_v = attw_sb[:].rearrange("p (a four) -> p a four", four=4)
            ew_v = ew_sb[:].rearrange("p (a four) -> p a four", four=4)
            for beta in range(4):
                nc.scalar.activation(
                    out=ew_v[:, :, beta],
                    in_=attw_v[:, :, beta],
                    func=mybir.ActivationFunctionType.Exp,
                    accum_out=z_sb[:, beta : beta + 1],
                )
            nc.vector.reciprocal(out=zi_sb[:], in_=z_sb[:])
            nc.vector.tensor_copy(out=prod_sb[:, 0:1], in_=ew_sb[:, 0:1])
            nc.vector.tensor_tensor(
                out=prod_sb[:], in0=ew_sb[:], in1=q_sb[:], op=mybir.AluOpType.mult
            )
            nc.vector.reduce_sum(
                out=s4_sb[:],
                in_=prod_sb[:].rearrange("p (a four) -> p four a", four=4),
                axis=mybir.AxisListType.X,
            )
            nc.vector.tensor_tensor(
                out=fm_sb[:], in0=s4_sb[:], in1=zi_sb[:], op=mybir.AluOpType.mult
            )
            # out[b], b = blk*128 + part; fm bucket beta = 2*(blk%2) + blk//2
            otrig = [nc.scalar, nc.gpsimd, nc.sync, nc.gpsimd]
            for blk in range(4):
                beta = 2 * (blk % 2) + blk // 2
                nc.vector.tensor_add(
                    out=out_sb[:, blk : blk + 1],
                    in0=lino_sb[:, blk : blk + 1],
                    in1=fm_sb[:, beta : beta + 1],
                )
                otrig[blk].dma_start(
                    out=out_d[blk * 128 : (blk + 1) * 128],
                    in_=out_sb[:, blk : blk + 1],
                )
    if split_waits:
        _split_excess_waits(nc)
    # populate .instr bytes for extended insts — raw Bass skips this Bacc
    # pass; without it walrus fails "ISA wrong length"
    mybir.codegen_inst_isa_subclasses(nc)
    return nc


def prep_inputs(sparse_feat, dense_feat, embed_table, lin_table, w_dense, bias, W1, b1, w2, p):
    """Host-side preprocessing -> per-core in_maps."""
    sparse_feat = np.asarray(sparse_feat)
    dense_feat = np.asarray(dense_feat, dtype=np.float32)
    embed_table = np.asarray(embed_table, dtype=np.float32)
    lin_table = np.asarray(lin_table, dtype=np.float32)
    w_dense = np.asarray(w_dense, dtype=np.float32)
    bias = np.asarray(bias, dtype=np.float32)
    W1 = np.asarray(W1, dtype=np.float32)
    b1 = np.asarray(b1, dtype=np.float32)
    w2 = np.asarray(w2, dtype=np.float32)
    p = np.asarray(p, dtype=np.float32)
    bf = ml_dtypes.bfloat16
    emb_bf = (embed_table * EMB_SCALE).astype(bf)
    lin_f32 = lin_table.astype(np.float32)

    w1blk = np.zeros((128, 128), dtype=bf)
    w1blk[:D, :D] = W1.astype(bf)
    w1blk[D:, D:] = W1.astype(bf)
    w2blk = np.zeros((128, 2), dtype=bf)
    w2blk[:D, 0] = w2.astype(bf)
    w2blk[D:, 1] = w2.astype(bf)
    pblk = np.zeros((128, 2), dtype=bf)
    pblk[:D, 0] = p.astype(bf)
    pblk[D:, 1] = p.astype(bf)
    b1c = np.tile(b1.astype(np.float32), 2).reshape(128, 1)
    wd14 = np.tile(np.concatenate([w_dense, bias[:1]]).astype(np.float32), (128, 1))

    in_maps = []
    for c in range(NCORES):
        sl = slice(c * BC, (c + 1) * BC)
        sf = np.asarray(sparse_feat[sl], dtype=np.int64)  # [512, 26]
        # host-side gather + transpose: embT[d + 64*g, f*256 + (b%256)]
        g = emb_bf[sf]  # [512, 26, 64]
        embt = np.ascontiguousarray(
            g.reshape(2, 256, F, D).transpose(0, 3, 2, 1).reshape(128, F * 256)
        )
        # lin values: [128, (blk f)] with b = blk*128 + p
        lv = lin_f32[sf]  # [512, 26] f32
        linarr = np.ascontiguousarray(
            lv.reshape(4, 128, F).transpose(1, 0, 2).reshape(128, 4 * F)
        )
        df = np.asarray(dense_feat[sl], dtype=np.float32)
        d14 = np.concatenate([df, np.ones((BC, 1), np.float32)], axis=1)
        d14 = d14.reshape(4, 128, 14).transpose(1, 0, 2).reshape(128, 4 * 14)
        d14 = np.concatenate([d14, wd14], axis=1).copy()  # [128, 70]
        in_maps.append(
            {
                "embt": embt,
                "lin": linarr,
                "dense": d14,
                "w1blk": w1blk,
                "b1c": b1c,
                "w2blk": w2blk,
                "pblk": pblk,
            }
        )
    return in_maps


_NC_CACHE = {}


def kernel(**inputs):
    if "nc" not in _NC_CACHE:
        _NC_CACHE["nc"] = build_nc()
    nc = _NC_CACHE["nc"]
    in_maps = prep_inputs(**inputs)
    res = run_bass_kernel_spmd(nc, in_maps, core_ids=list(range(NCORES)))
    outs = [np.asarray(res.results[c]["out"], dtype=np.float32) for c in range(NCORES)]
    return np.concatenate(outs, axis=0)


# revision 17
# speedup vs baseline: 1.2745x; 1.2745x over previous
"""AFM (Attentional Factorization Machine) Trainium2 kernel, 8-core data-parallel.

Reference computation (per batch row b):
  emb        = embed_table[sparse_feat[b]]                      [26, 64]
  linear_out = sum_f lin_table[sparse_feat[b,f]] + dense.w_d + bias
  att_x[p]   = emb[i_p] * emb[j_p]          (325 pairs i<j)     [325, 64]
  h          = relu(att_x @ W1 + b1);  att_w = h @ w2           [325]
  score      = softmax(att_w);  out = linear_out + (sum_p score_p att_x[p]) . p

Strategy: shard batch (4096 -> 512/core).  Host prep does the embedding
gather AND the transpose, shipping a ready-to-use embT per core:
  embT [128 = d + 64*g, 26*256], g = b//256 (two batch-groups packed on
  partitions), col = f*256 + (b%256).  Device just DMAs it (1.7MB,
  split into 7 four-field parts so chunk compute starts as soon as the
  first fields land).
Per core:
  - pairs processed j-major: chunk j covers pairs (i<j) -> att_x chunk via DVE
    tensor_tensor with step-0 broadcast of emb_j over the i-run, bf16.
  - mm1: lhsT = blockdiag(W1,W1) [128,128], stream attx -> h psum [128,1024].
  - relu+bias via ACT (one op per 1024 cols) -> relu_h bf16.
  - att_w: per 128-col tile, matmul(lhsT=relu_h_tile, rhs=[[w2,0],[0,w2]]) ->
    psum [128 b, 2];  q likewise from attx with [[p],[p]].  b-on-partitions.
  - softmax over pairs without max subtraction (logits are tiny); exp+sum
    fused on ACT via accum_out; out = linear + (sum ew*q)/Z.
"""

import os
import sys

sys.path.insert(0, "/opt/trn_rl_repo")

import numpy as np
import ml_dtypes

from concourse import bass, library_config, mybir, tile
from concourse import bass_utils as _bu
from concourse.bass_utils import run_bass_kernel_spmd
from concourse.masks import make_identity

BF16 = mybir.dt.bfloat16
FP8 = mybir.dt.float8e4
F32 = mybir.dt.float32
I32 = mybir.dt.int32

# emb is pre-scaled x16 on host so h/rh (x256) sit in fp8 e4m3's normal
# range; att_w/q are unscaled (x1/256) on psum readout.
EMB_SCALE = 16.0
UNSCALE = 1.0 / (EMB_SCALE * EMB_SCALE)

V = 100000
F = 26
D = 64
BC = 512  # batch per core
NCORES = 8
NPAIR = F * (F - 1) // 2  # 325
NPART = (F + 3) // 4  # 7 four-field parts


def _split_excess_waits(nc, cap=1):
    """This walrus build gives most instruction structs a single embedded
    sync-wait slot; Tile can emit more.  Hoist extra waits onto injected
    same-engine NoOps right before the instruction (engine FIFO makes the
    combined wait-set semantics identical)."""
    skip = {"EventSemaphore", "AllEngineBarrier", "UnconditionalBranch",
            "CompareAndBranch", "BranchHint"}
    for fn in nc.m.functions:
        for blk in fn.blocks:
            new_insts = []
            for inst in blk.instructions:
                if inst.opcode == "Ldweights" and inst.tile_position == (0, 0):
                    # a (0,0)/full tile_position is a no-op but trips walrus's
                    # "not compatible with LDW optimization" check
                    inst.tile_position = None
                    inst.tile_size = None
                si = inst.sync_info
                if (
                    si is not None
                    and si.on_wait
                    and len(si.on_wait) > cap
                    and inst.opcode not in skip
                ):
                    waits = list(si.on_wait)
                    for w in waits[:-cap]:
                        nop = mybir.InstNoOp(
                            name=nc.get_next_instruction_name(),
                            engine=inst.engine,
                            bass_nofuse=True,
                            sync_info=mybir.SyncInfo(on_wait=[w], on_update=[]),
                        )
                        new_insts.append(nop)
                    si.on_wait = waits[-cap:]
                new_insts.append(inst)
            blk.instructions = new_insts
    return nc


def build_nc(split_waits=True):
    nc = bass.Bass()

    embt_d = nc.declare_dram_parameter("embt", [128, F * 256], BF16, isOutput=False)
    lin_d = nc.declare_dram_parameter("lin", [128, 4 * F], F32, isOutput=False)
    dense = nc.declare_dram_parameter("dense", [128, 70], F32, isOutput=False)
    w1blk = nc.declare_dram_parameter("w1blk", [128, 128], FP8, isOutput=False)
    b1c = nc.declare_dram_parameter("b1c", [128, 1], F32, isOutput=False)
    w2blk = nc.declare_dram_parameter("w2blk", [128, 2], FP8, isOutput=False)
    pblk = nc.declare_dram_parameter("pblk", [128, 2], BF16, isOutput=False)
    out_d = nc.declare_dram_parameter("out", [BC], F32, isOutput=True)

    NTILE = 2 * NPAIR  # 650 column-tiles of 128 (pair x bhalf)
    NWCOL = 2 * NTILE  # 1300 psum output cols (x2 groups)

    with tile.TileContext(nc) as tc:
        with (
            tc.tile_pool(name="const", bufs=1) as cpool,
            tc.tile_pool(name="ax", bufs=3) as axpool,
            tc.tile_pool(name="rh", bufs=4) as rhpool,
            tc.tile_pool(name="ps", bufs=2, space="PSUM") as pspool,
        ):
            # ---- persistent sbuf tensors ----
            embt_p = [
                cpool.tile(
                    [128, min(4, F - 4 * pp) * 256], BF16,
                    tag=f"embt{pp}", name=f"embt{pp}",
                )
                for pp in range(NPART)
            ]
            lin_sb = cpool.tile([128, 4 * F], F32, tag="lin")
            densewd_sb = cpool.tile([128, 70], F32, tag="dense")
            dense_sb = densewd_sb[:, 0:56]
            wd_sb = densewd_sb[:, 56:70]
            w1_sb = cpool.tile([128, 128], FP8, tag="w1")
            b1_sb = cpool.tile([128, 1], F32, tag="b1")
            w2_sb = cpool.tile([128, 2], FP8, tag="w2")
            p_sb = cpool.tile([128, 2], BF16, tag="p")
            ident = cpool.tile([128, 128], BF16, tag="ident")
            attw_sb = cpool.tile([128, NWCOL], F32, tag="attw")
            q_sb = cpool.tile([128, NWCOL], F32, tag="q")
            ew_sb = cpool.tile([128, NWCOL], F32, tag="ew")
            prod_sb = cpool.tile([128, NWCOL], F32, tag="prod")
            lsum_sb = cpool.tile([128, 4], F32, tag="lsum")
            dtmp_sb = cpool.tile([128, 4 * 14], F32, tag="dtmp")
            dsum_sb = cpool.tile([128, 4], F32, tag="dsum")
            lino_sb = cpool.tile([128, 4], F32, tag="lino")
            z_sb = cpool.tile([128, 4], F32, tag="z")
            zi_sb = cpool.tile([128, 4], F32, tag="zi")
            s4_sb = cpool.tile([128, 4], F32, tag="s4")
            fm_sb = cpool.tile([128, 4], F32, tag="fm")
            out_sb = cpool.tile([128, 4], F32, tag="outsb")

            # ---- load parameters ----
            # Small params first (their triggers are fast and W1/b1 gate the
            # first matmul/relu); embt triggers spread across the other
            # engine sequencers so the ~600ns DIRECT2D issues parallelize.
            nc.sync.dma_start(out=w1_sb[:], in_=w1blk[:, :])
            nc.sync.dma_start(out=b1_sb[:], in_=b1c[:, :])
            nc.sync.dma_start(out=w2_sb[:], in_=w2blk[:, :])
            nc.sync.dma_start(out=p_sb[:], in_=pblk[:, :])
            trig = [nc.scalar, nc.gpsimd, nc.sync]
            ti = 0
            for pp in range(NPART):
                nf = min(4, F - 4 * pp)
                # split the first parts across partition halves so the
                # earliest chunks' inputs land ~2x sooner
                rsplit = [(0, 64), (64, 128)] if pp < 3 else [(0, 128)]
                for r0, r1 in rsplit:
                    trig[ti % 3].dma_start(
                        out=embt_p[pp][r0:r1, :],
                        in_=embt_d[r0:r1, pp * 1024 : pp * 1024 + nf * 256],
                    )
                    ti += 1
            nc.gpsimd.dma_start(out=lin_sb[:], in_=lin_d[:, :])
            nc.sync.dma_start(out=densewd_sb[:], in_=dense[:, :])
            make_identity(nc, ident[:])

            # ---- chunk j: pairs (i, j) for i < j ----
            state = {
                "tdone": 0, "copied": 0, "ps_w": None, "ps_q": None,
                "gidx": 0, "axidx": 0, "pend": [],
            }

            def emit_mm2s(rh, ax, n0, N):
                """Second-stage matmuls for one 1024-col group (emitted one
                group late so PE never waits on the relu of the group it is
                currently streaming)."""
                for t0 in range(0, N, 128):
                    u = state["tdone"] % 256
                    if u == 0:
                        state["ps_w"] = pspool.tile(
                            [128, 512], F32, tag="w", name=f"psw{state['tdone']}"
                        )
                        state["ps_q"] = pspool.tile(
                            [128, 512], F32, tag="q", name=f"psq{state['tdone']}"
                        )
                    # q first: it depends only on ax, not on the relu output
                    nc.tensor.matmul(
                        out=state["ps_q"][:, 2 * u : 2 * u + 2],
                        lhsT=ax[:, n0 + t0 : n0 + t0 + 128],
                        rhs=p_sb[:],
                        start=True,
                        stop=True,
                    )
                    nc.tensor.matmul(
                        out=state["ps_w"][:, 2 * u : 2 * u + 2],
                        lhsT=rh[:, t0 : t0 + 128],
                        rhs=w2_sb[:],
                        start=True,
                        stop=True,
                    )
                    state["tdone"] += 1
                    if state["tdone"] % 256 == 0 or state["tdone"] == NTILE:
                        w2c = 2 * ((state["tdone"] - 1) % 256 + 1)
                        c0 = state["copied"]
                        nc.vector.tensor_scalar(
                            out=attw_sb[:, c0 : c0 + w2c],
                            in0=state["ps_w"][:, :w2c],
                            scalar1=UNSCALE,
                            scalar2=None,
                            op0=mybir.AluOpType.mult,
                        )
                        nc.vector.tensor_scalar(
                            out=q_sb[:, c0 : c0 + w2c],
                            in0=state["ps_q"][:, :w2c],
                            scalar1=UNSCALE,
                            scalar2=None,
                            op0=mybir.AluOpType.mult,
                        )
                        state["copied"] += w2c

            def emit_chunk(j):
                jpp, jo = j // 4, j % 4
                ax = axpool.tile([128, 25 * 256], BF16, tag="ax", name=f"ax{j}")
                # absorb WAR-on-slot (PE readers) into a 1-wait-capable copy
                nc.vector.tensor_copy(out=ax[:, 0:1], in_=ident[:, 0:1])
                in0 = embt_p[jpp][:, jo * 256 : (jo + 1) * 256].rearrange(
                    "p (one w) -> p one w", one=1
                )
                for ib in range(0, j, 4):
                    w = min(4, j - ib)
                    # NOTE: gpsimd elementwise was tried here and abandoned:
                    # concurrent Q7 SBUF traffic slows DVE ops ~3x.
                    axeng = nc.vector
                    axeng.tensor_tensor(
                        out=ax[:, ib * 256 : (ib + w) * 256].rearrange(
                            "p (l w) -> p l w", l=w
                        ),
                        in0=in0.to_broadcast([128, w, 256]),
                        in1=embt_p[ib // 4][:, : w * 256].rearrange(
                            "p (l w) -> p l w", l=w
                        ),
                        op=mybir.AluOpType.mult,
                    )
                ncols = j * 256
                for n0 in range(0, ncols, 1024):
                    N = min(1024, ncols - n0)
                    hp = pspool.tile([128, 1024], F32, tag="h", name=f"hp{j}_{n0}")
                    for m0 in range(0, N, 512):
                        M = min(512, N - m0)
                        nc.tensor.matmul(
                            out=hp[:, m0 : m0 + M],
                            lhsT=w1_sb[:],
                            rhs=ax[:, n0 + m0 : n0 + m0 + M],
                            start=True,
                            stop=True,
                        )
                    rh = rhpool.tile([128, 1024], FP8, tag="rh", name=f"rh{j}_{n0}")
                    # split relu 4:1 between ACT and DVE
                    if state["gidx"] % 5 < 4:
                        nc.scalar.activation(
                            out=rh[:, :N],
                            in_=hp[:, :N],
                            func=mybir.ActivationFunctionType.Relu,
                            bias=b1_sb[:],
                        )
                    else:
                        nc.vector.tensor_scalar(
                            out=rh[:, :N],
                            in0=hp[:, :N],
                            scalar1=b1_sb[:, 0:1],
                            scalar2=0.0,
                            op0=mybir.AluOpType.add,
                            op1=mybir.AluOpType.max,
                        )
                    state["gidx"] += 1
                    state["pend"].append((rh, ax, n0, N))
                    if len(state["pend"]) > 2:
                        emit_mm2s(*state["pend"].pop(0))

            # Order: small chunks first while embT streams in, big chunks
            # mid-kernel, tiny chunks last so the MM2/relu drain is short.
            order = [1, 2, 3, 7, 8, 9, 10, 11] + list(range(25, 11, -1)) + [4, 6, 5]
            assert sorted(order) == list(range(1, F))
            for j in order:
                emit_chunk(j)
            while state["pend"]:
                emit_mm2s(*state["pend"].pop(0))

            # ---- linear tail ----
            nc.vector.reduce_sum(
                out=lsum_sb[:],
                in_=lin_sb[:].rearrange("p (blk f) -> p blk f", blk=4),
                axis=mybir.AxisListType.X,
            )
            for blk in range(4):
                nc.vector.tensor_tensor(
                    out=dtmp_sb[:, blk * 14 : (blk + 1) * 14],
                    in0=dense_sb[:, blk * 14 : (blk + 1) * 14],
                    in1=wd_sb[:, :],
                    op=mybir.AluOpType.mult,
                )
            nc.vector.reduce_sum(
                out=dsum_sb[:],
                in_=dtmp_sb[:].rearrange("p (blk w) -> p blk w", blk=4),
                axis=mybir.AxisListType.X,
            )
            nc.vector.tensor_add(out=lino_sb[:], in0=lsum_sb[:], in1=dsum_sb[:])

            # ---- softmax (no max subtraction; logits are tiny) ----
            attw_v = attw_sb[:].rearrange("p (a four) -> p a four", four=4)
            ew_v = ew_sb[:].rearrange("p (a four) -> p a four", four=4)
            for beta in range(4):
                nc.scalar.activation(
                    out=ew_v[:, :, beta],
                    in_=attw_v[:, :, beta],
                    func=mybir.ActivationFunctionType.Exp,
                    accum_out=z_sb[:, beta : beta + 1],
                )
            nc.vector.reciprocal(out=zi_sb[:], in_=z_sb[:])
            nc.vector.tensor_copy(out=prod_sb[:, 0:1], in_=ew_sb[:, 0:1])
            nc.vector.tensor_tensor(
                out=prod_sb[:], in0=ew_sb[:], in1=q_sb[:], op=mybir.AluOpType.mult
            )
            nc.vector.reduce_sum(
                out=s4_sb[:],
                in_=prod_sb[:].rearrange("p (a four) -> p four a", four=4),
                axis=mybir.AxisListType.X,
            )
            nc.vector.tensor_tensor(
                out=fm_sb[:], in0=s4_sb[:], in1=zi_sb[:], op=mybir.AluOpType.mult
            )
            # out[b], b = blk*128 + part; fm bucket beta = 2*(blk%2) + blk//2
            otrig = [nc.scalar, nc.gpsimd, nc.sync, nc.gpsimd]
            for blk in range(4):
                beta = 2 * (blk % 2) + blk // 2
                nc.vector.tensor_add(
                    out=out_sb[:, blk : blk + 1],
                    in0=lino_sb[:, blk : blk + 1],
                    in1=fm_sb[:, beta : beta + 1],
                )
                otrig[blk].dma_start(
                    out=out_d[blk * 128 : (blk + 1) * 128],
                    in_=out_sb[:, blk : blk + 1],
                )
    if split_waits:
        _split_excess_waits(nc)
    # populate .instr bytes for extended insts — raw Bass skips this Bacc
    # pass; without it walrus fails "ISA wrong length"
    mybir.codegen_inst_isa_subclasses(nc)
    return nc


def prep_inputs(sparse_feat, dense_feat, embed_table, lin_table, w_dense, bias, W1, b1, w2, p):
    """Host-side preprocessing -> per-core in_maps."""
    sparse_feat = np.asarray(sparse_feat)
    dense_feat = np.asarray(dense_feat, dtype=np.float32)
    embed_table = np.asarray(embed_table, dtype=np.float32)
    lin_table = np.asarray(lin_table, dtype=np.float32)
    w_dense = np.asarray(w_dense, dtype=np.float32)
    bias = np.asarray(bias, dtype=np.float32)
    W1 = np.asarray(W1, dtype=np.float32)
    b1 = np.asarray(b1, dtype=np.float32)
    w2 = np.asarray(w2, dtype=np.float32)
    p = np.asarray(p, dtype=np.float32)
    bf = ml_dtypes.bfloat16
    emb_bf = (embed_table * EMB_SCALE).astype(bf)
    lin_f32 = lin_table.astype(np.float32)

    f8 = ml_dtypes.float8_e4m3
    w1blk = np.zeros((128, 128), dtype=f8)
    w1blk[:D, :D] = W1.astype(f8)
    w1blk[D:, D:] = W1.astype(f8)
    w2blk = np.zeros((128, 2), dtype=f8)
    w2blk[:D, 0] = w2.astype(f8)
    w2blk[D:, 1] = w2.astype(f8)
    pblk = np.zeros((128, 2), dtype=bf)
    pblk[:D, 0] = p.astype(bf)
    pblk[D:, 1] = p.astype(bf)
    # h in psum carries EMB_SCALE^2; bias must match
    b1c = np.tile((b1 * EMB_SCALE * EMB_SCALE).astype(np.float32), 2).reshape(128, 1)
    wd14 = np.tile(np.concatenate([w_dense, bias[:1]]).astype(np.float32), (128, 1))

    in_maps = []
    for c in range(NCORES):
        sl = slice(c * BC, (c + 1) * BC)
        sf = np.asarray(sparse_feat[sl], dtype=np.int64)  # [512, 26]
        # host-side gather + transpose: embT[d + 64*g, f*256 + (b%256)]
        g = emb_bf[sf]  # [512, 26, 64]
        embt = np.ascontiguousarray(
            g.reshape(2, 256, F, D).transpose(0, 3, 2, 1).reshape(128, F * 256)
        )
        # lin values: [128, (blk f)] with b = blk*128 + p
        lv = lin_f32[sf]  # [512, 26] f32
        linarr = np.ascontiguousarray(
            lv.reshape(4, 128, F).transpose(1, 0, 2).reshape(128, 4 * F)
        )
        df = np.asarray(dense_feat[sl], dtype=np.float32)
        d14 = np.concatenate([df, np.ones((BC, 1), np.float32)], axis=1)
        d14 = d14.reshape(4, 128, 14).transpose(1, 0, 2).reshape(128, 4 * 14)
        d14 = np.concatenate([d14, wd14], axis=1).copy()  # [128, 70]
        in_maps.append(
            {
                "embt": embt,
                "lin": linarr,
                "dense": d14,
                "w1blk": w1blk,
                "b1c": b1c,
                "w2blk": w2blk,
                "pblk": pblk,
            }
        )
    return in_maps


_NC_CACHE = {}


def kernel(**inputs):
    if "nc" not in _NC_CACHE:
        _NC_CACHE["nc"] = build_nc()
    nc = _NC_CACHE["nc"]
    in_maps = prep_inputs(**inputs)
    res = run_bass_kernel_spmd(nc, in_maps, core_ids=list(range(NCORES)))
    outs = [np.asarray(res.results[c]["out"], dtype=np.float32) for c in range(NCORES)]
    return np.concatenate(outs, axis=0)


# revision 20
# speedup vs baseline: 1.3291x; 1.0429x over previous
"""AFM (Attentional Factorization Machine) Trainium2 kernel, 8-core data-parallel.

Reference computation (per batch row b):
  emb        = embed_table[sparse_feat[b]]                      [26, 64]
  linear_out = sum_f lin_table[sparse_feat[b,f]] + dense.w_d + bias
  att_x[p]   = emb[i_p] * emb[j_p]          (325 pairs i<j)     [325, 64]
  h          = relu(att_x @ W1 + b1);  att_w = h @ w2           [325]
  score      = softmax(att_w);  out = linear_out + (sum_p score_p att_x[p]) . p

Strategy: shard batch (4096 -> 512/core).  Host prep does the embedding
gather AND the transpose, shipping a ready-to-use embT per core:
  embT [128 = d + 64*g, 26*256], g = b//256 (two batch-groups packed on
  partitions), col = f*256 + (b%256).  Device just DMAs it (1.7MB,
  split into 7 four-field parts so chunk compute starts as soon as the
  first fields land).
Per core:
  - pairs processed j-major: chunk j covers pairs (i<j) -> att_x chunk via DVE
    tensor_tensor with step-0 broadcast of emb_j over the i-run, bf16.
  - mm1: lhsT = blockdiag(W1,W1) [128,128], stream attx -> h psum [128,1024].
  - relu+bias via ACT (one op per 1024 cols) -> relu_h bf16.
  - att_w: per 128-col tile, matmul(lhsT=relu_h_tile, rhs=[[w2,0],[0,w2]]) ->
    psum [128 b, 2];  q likewise from attx with [[p],[p]].  b-on-partitions.
  - softmax over pairs without max subtraction (logits are tiny); exp+sum
    fused on ACT via accum_out; out = linear + (sum ew*q)/Z.
"""

import os
import sys

sys.path.insert(0, "/opt/trn_rl_repo")

import numpy as np
import ml_dtypes

from concourse import bass, library_config, mybir, tile
from concourse import bass_utils as _bu
from concourse.bass_utils import run_bass_kernel_spmd
from concourse.masks import make_identity

BF16 = mybir.dt.bfloat16
FP8 = mybir.dt.float8e4
F32 = mybir.dt.float32
I32 = mybir.dt.int32

# emb is pre-scaled x16 on host so h/rh (x256) sit in fp8 e4m3's normal
# range; att_w/q are unscaled (x1/256) on psum readout.
EMB_SCALE = 16.0
UNSCALE = 1.0 / (EMB_SCALE * EMB_SCALE)

V = 100000
F = 26
D = 64
BC = 512  # batch per core
NCORES = 8
NPAIR = F * (F - 1) // 2  # 325
NPART = (F + 3) // 4  # 7 four-field parts


def _split_excess_waits(nc, cap=1):
    """This walrus build gives most instruction structs a single embedded
    sync-wait slot; Tile can emit more.  Hoist extra waits onto injected
    same-engine NoOps right before the instruction (engine FIFO makes the
    combined wait-set semantics identical)."""
    skip = {"EventSemaphore", "AllEngineBarrier", "UnconditionalBranch",
            "CompareAndBranch", "BranchHint"}
    for fn in nc.m.functions:
        for blk in fn.blocks:
            new_insts = []
            for inst in blk.instructions:
                if inst.opcode == "Ldweights" and inst.tile_position == (0, 0):
                    # a (0,0)/full tile_position is a no-op but trips walrus's
                    # "not compatible with LDW optimization" check
                    inst.tile_position = None
                    inst.tile_size = None
                si = inst.sync_info
                if (
                    si is not None
                    and si.on_wait
                    and len(si.on_wait) > cap
                    and inst.opcode not in skip
                ):
                    waits = list(si.on_wait)
                    for w in waits[:-cap]:
                        nop = mybir.InstNoOp(
                            name=nc.get_next_instruction_name(),
                            engine=inst.engine,
                            bass_nofuse=True,
                            sync_info=mybir.SyncInfo(on_wait=[w], on_update=[]),
                        )
                        new_insts.append(nop)
                    si.on_wait = waits[-cap:]
                new_insts.append(inst)
            blk.instructions = new_insts
    return nc


def build_nc(split_waits=True):
    nc = bass.Bass()

    embt_d = nc.declare_dram_parameter("embt", [128, F * 256], BF16, isOutput=False)
    lin_d = nc.declare_dram_parameter("lin", [128, 4 * F], F32, isOutput=False)
    dense = nc.declare_dram_parameter("dense", [128, 70], F32, isOutput=False)
    w1blk = nc.declare_dram_parameter("w1blk", [128, 128], FP8, isOutput=False)
    b1c = nc.declare_dram_parameter("b1c", [128, 1], F32, isOutput=False)
    w2blk = nc.declare_dram_parameter("w2blk", [128, 2], FP8, isOutput=False)
    pblk = nc.declare_dram_parameter("pblk", [128, 2], BF16, isOutput=False)
    out_d = nc.declare_dram_parameter("out", [BC], F32, isOutput=True)

    NTILE = 2 * NPAIR  # 650 column-tiles of 128 (pair x bhalf)
    NWCOL = 2 * NTILE  # 1300 psum output cols (x2 groups)

    with tile.TileContext(nc) as tc:
        with (
            tc.tile_pool(name="const", bufs=1) as cpool,
            tc.tile_pool(name="ax", bufs=3) as axpool,
            tc.tile_pool(name="rh", bufs=4) as rhpool,
            tc.tile_pool(name="ps", bufs=2, space="PSUM") as pspool,
        ):
            # ---- persistent sbuf tensors ----
            embt_p = [
                cpool.tile(
                    [128, min(4, F - 4 * pp) * 256], BF16,
                    tag=f"embt{pp}", name=f"embt{pp}",
                )
                for pp in range(NPART)
            ]
            lin_sb = cpool.tile([128, 4 * F], F32, tag="lin")
            densewd_sb = cpool.tile([128, 70], F32, tag="dense")
            dense_sb = densewd_sb[:, 0:56]
            wd_sb = densewd_sb[:, 56:70]
            w1_sb = cpool.tile([128, 128], FP8, tag="w1")
            b1_sb = cpool.tile([128, 1], F32, tag="b1")
            w2_sb = cpool.tile([128, 2], FP8, tag="w2")
            p_sb = cpool.tile([128, 2], BF16, tag="p")
            ident = cpool.tile([128, 128], BF16, tag="ident")
            attw_sb = cpool.tile([128, NWCOL], F32, tag="attw")
            q_sb = cpool.tile([128, NWCOL], F32, tag="q")
            ew_sb = cpool.tile([128, NWCOL], F32, tag="ew")
            prod_sb = cpool.tile([128, NWCOL], F32, tag="prod")
            lsum_sb = cpool.tile([128, 4], F32, tag="lsum")
            dtmp_sb = cpool.tile([128, 4 * 14], F32, tag="dtmp")
            dsum_sb = cpool.tile([128, 4], F32, tag="dsum")
            lino_sb = cpool.tile([128, 4], F32, tag="lino")
            z_sb = cpool.tile([128, 4], F32, tag="z")
            zi_sb = cpool.tile([128, 4], F32, tag="zi")
            s4_sb = cpool.tile([128, 4], F32, tag="s4")
            fm_sb = cpool.tile([128, 4], F32, tag="fm")
            out_sb = cpool.tile([128, 4], F32, tag="outsb")

            # ---- load parameters ----
            # Small params first (their triggers are fast and W1/b1 gate the
            # first matmul/relu); embt triggers spread across the other
            # engine sequencers so the ~600ns DIRECT2D issues parallelize.
            nc.sync.dma_start(out=w1_sb[:], in_=w1blk[:, :])
            nc.sync.dma_start(out=b1_sb[:], in_=b1c[:, :])
            nc.sync.dma_start(out=w2_sb[:], in_=w2blk[:, :])
            nc.sync.dma_start(out=p_sb[:], in_=pblk[:, :])
            trig = [nc.scalar, nc.gpsimd, nc.sync]
            for pp in range(NPART):
                nf = min(4, F - 4 * pp)
                trig[pp % 3].dma_start(
                    out=embt_p[pp][:],
                    in_=embt_d[:, pp * 1024 : pp * 1024 + nf * 256],
                )
            nc.gpsimd.dma_start(out=lin_sb[:], in_=lin_d[:, :])
            nc.sync.dma_start(out=densewd_sb[:], in_=dense[:, :])
            make_identity(nc, ident[:])

            # ---- chunk j: pairs (i, j) for i < j ----
            state = {
                "tdone": 0, "copied": 0, "ps_w": None, "ps_q": None,
                "gidx": 0, "axidx": 0, "pend": [],
            }

            def emit_mm2s(rh, ax, n0, N):
                """Second-stage matmuls for one group (emitted two groups
                late so PE never waits on the relu of the group it is
                currently streaming).  w and q share one psum bank."""
                for t0 in range(0, N, 128):
                    u = state["tdone"] % 128
                    if u == 0:
                        state["ps_w"] = pspool.tile(
                            [128, 512], F32, tag="w", name=f"psw{state['tdone']}"
                        )
                    # q first: it depends only on ax, not on the relu output
                    nc.tensor.matmul(
                        out=state["ps_w"][:, 256 + 2 * u : 256 + 2 * u + 2],
                        lhsT=ax[:, n0 + t0 : n0 + t0 + 128],
                        rhs=p_sb[:],
                        start=True,
                        stop=True,
                    )
                    nc.tensor.matmul(
                        out=state["ps_w"][:, 2 * u : 2 * u + 2],
                        lhsT=rh[:, t0 : t0 + 128],
                        rhs=w2_sb[:],
                        start=True,
                        stop=True,
                    )
                    state["tdone"] += 1
                    if state["tdone"] % 128 == 0 or state["tdone"] == NTILE:
                        w2c = 2 * ((state["tdone"] - 1) % 128 + 1)
                        c0 = state["copied"]
                        nc.vector.tensor_scalar(
                            out=attw_sb[:, c0 : c0 + w2c],
                            in0=state["ps_w"][:, :w2c],
                            scalar1=UNSCALE,
                            scalar2=None,
                            op0=mybir.AluOpType.mult,
                        )
                        nc.vector.tensor_scalar(
                            out=q_sb[:, c0 : c0 + w2c],
                            in0=state["ps_w"][:, 256 : 256 + w2c],
                            scalar1=UNSCALE,
                            scalar2=None,
                            op0=mybir.AluOpType.mult,
                        )
                        state["copied"] += w2c

            def emit_chunk(j):
                jpp, jo = j // 4, j % 4
                ax = axpool.tile([128, 25 * 256], BF16, tag="ax", name=f"ax{j}")
                # absorb WAR-on-slot (PE readers) into a 1-wait-capable copy
                nc.vector.tensor_copy(out=ax[:, 0:1], in_=ident[:, 0:1])
                in0 = embt_p[jpp][:, jo * 256 : (jo + 1) * 256].rearrange(
                    "p (one w) -> p one w", one=1
                )
                for ib in range(0, j, 4):
                    w = min(4, j - ib)
                    # NOTE: gpsimd elementwise was tried here and abandoned:
                    # concurrent Q7 SBUF traffic slows DVE ops ~3x.
                    axeng = nc.vector
                    axeng.tensor_tensor(
                        out=ax[:, ib * 256 : (ib + w) * 256].rearrange(
                            "p (l w) -> p l w", l=w
                        ),
                        in0=in0.to_broadcast([128, w, 256]),
                        in1=embt_p[ib // 4][:, : w * 256].rearrange(
                            "p (l w) -> p l w", l=w
                        ),
                        op=mybir.AluOpType.mult,
                    )
                ncols = j * 256
                for n0 in range(0, ncols, 1536):
                    N = min(1536, ncols - n0)
                    hp = pspool.tile([128, 1536], F32, tag="h", name=f"hp{j}_{n0}")
                    for m0 in range(0, N, 512):
                        M = min(512, N - m0)
                        nc.tensor.matmul(
                            out=hp[:, m0 : m0 + M],
                            lhsT=w1_sb[:],
                            rhs=ax[:, n0 + m0 : n0 + m0 + M],
                            start=True,
                            stop=True,
                        )
                    rh = rhpool.tile([128, 1536], FP8, tag="rh", name=f"rh{j}_{n0}")
                    # split relu 4:1 between ACT and DVE
                    if state["gidx"] % 5 < 4:
                        nc.scalar.activation(
                            out=rh[:, :N],
                            in_=hp[:, :N],
                            func=mybir.ActivationFunctionType.Relu,
                            bias=b1_sb[:],
                        )
                    else:
                        nc.vector.tensor_scalar(
                            out=rh[:, :N],
                            in0=hp[:, :N],
                            scalar1=b1_sb[:, 0:1],
                            scalar2=0.0,
                            op0=mybir.AluOpType.add,
                            op1=mybir.AluOpType.max,
                        )
                    state["gidx"] += 1
                    state["pend"].append((rh, ax, n0, N))
                    if len(state["pend"]) > 2:
                        emit_mm2s(*state["pend"].pop(0))

            # Order: small chunks first while embT streams in, big chunks
            # mid-kernel, tiny chunks last so the MM2/relu drain is short.
            order = [1, 2, 3, 7, 8, 9, 10, 11] + list(range(25, 11, -1)) + [4, 6, 5]
            assert sorted(order) == list(range(1, F))
            for j in order:
                emit_chunk(j)
            while state["pend"]:
                emit_mm2s(*state["pend"].pop(0))

            # ---- linear tail ----
            nc.vector.reduce_sum(
                out=lsum_sb[:],
                in_=lin_sb[:].rearrange("p (blk f) -> p blk f", blk=4),
                axis=mybir.AxisListType.X,
            )
            for blk in range(4):
                nc.vector.tensor_tensor(
                    out=dtmp_sb[:, blk * 14 : (blk + 1) * 14],
                    in0=dense_sb[:, blk * 14 : (blk + 1) * 14],
                    in1=wd_sb[:, :],
                    op=mybir.AluOpType.mult,
                )
            nc.vector.reduce_sum(
                out=dsum_sb[:],
                in_=dtmp_sb[:].rearrange("p (blk w) -> p blk w", blk=4),
                axis=mybir.AxisListType.X,
            )
            nc.vector.tensor_add(out=lino_sb[:], in0=lsum_sb[:], in1=dsum_sb[:])

            # ---- softmax (no max subtraction; logits are tiny) ----
            attw_v = attw_sb[:].rearrange("p (a four) -> p a four", four=4)
            ew_v = ew_sb[:].rearrange("p (a four) -> p a four", four=4)
            for beta in range(4):
                nc.scalar.activation(
                    out=ew_v[:, :, beta],
                    in_=attw_v[:, :, beta],
                    func=mybir.ActivationFunctionType.Exp,
                    accum_out=z_sb[:, beta : beta + 1],
                )
            nc.vector.reciprocal(out=zi_sb[:], in_=z_sb[:])
            nc.vector.tensor_copy(out=prod_sb[:, 0:1], in_=ew_sb[:, 0:1])
            nc.vector.tensor_tensor(
                out=prod_sb[:], in0=ew_sb[:], in1=q_sb[:], op=mybir.AluOpType.mult
            )
            nc.vector.reduce_sum(
                out=s4_sb[:],
                in_=prod_sb[:].rearrange("p (a four) -> p four a", four=4),
                axis=mybir.AxisListType.X,
            )
            nc.vector.tensor_tensor(
                out=fm_sb[:], in0=s4_sb[:], in1=zi_sb[:], op=mybir.AluOpType.mult
            )
            # out[b], b = blk*128 + part; fm bucket beta = 2*(blk%2) + blk//2
            otrig = [nc.scalar, nc.gpsimd, nc.sync, nc.gpsimd]
            for blk in range(4):
                beta = 2 * (blk % 2) + blk // 2
                nc.vector.tensor_add(
                    out=out_sb[:, blk : blk + 1],
                    in0=lino_sb[:, blk : blk + 1],
                    in1=fm_sb[:, beta : beta + 1],
                )
                otrig[blk].dma_start(
                    out=out_d[blk * 128 : (blk + 1) * 128],
                    in_=out_sb[:, blk : blk + 1],
                )
    if split_waits:
        _split_excess_waits(nc)
    # populate .instr bytes for extended insts — raw Bass skips this Bacc
    # pass; without it walrus fails "ISA wrong length"
    mybir.codegen_inst_isa_subclasses(nc)
    return nc


def prep_inputs(sparse_feat, dense_feat, embed_table, lin_table, w_dense, bias, W1, b1, w2, p):
    """Host-side preprocessing -> per-core in_maps."""
    sparse_feat = np.asarray(sparse_feat)
    dense_feat = np.asarray(dense_feat, dtype=np.float32)
    embed_table = np.asarray(embed_table, dtype=np.float32)
    lin_table = np.asarray(lin_table, dtype=np.float32)
    w_dense = np.asarray(w_dense, dtype=np.float32)
    bias = np.asarray(bias, dtype=np.float32)
    W1 = np.asarray(W1, dtype=np.float32)
    b1 = np.asarray(b1, dtype=np.float32)
    w2 = np.asarray(w2, dtype=np.float32)
    p = np.asarray(p, dtype=np.float32)
    bf = ml_dtypes.bfloat16
    emb_bf = (embed_table * EMB_SCALE).astype(bf)
    lin_f32 = lin_table.astype(np.float32)

    f8 = ml_dtypes.float8_e4m3
    w1blk = np.zeros((128, 128), dtype=f8)
    w1blk[:D, :D] = W1.astype(f8)
    w1blk[D:, D:] = W1.astype(f8)
    w2blk = np.zeros((128, 2), dtype=f8)
    w2blk[:D, 0] = w2.astype(f8)
    w2blk[D:, 1] = w2.astype(f8)
    pblk = np.zeros((128, 2), dtype=bf)
    pblk[:D, 0] = p.astype(bf)
    pblk[D:, 1] = p.astype(bf)
    # h in psum carries EMB_SCALE^2; bias must match
    b1c = np.tile((b1 * EMB_SCALE * EMB_SCALE).astype(np.float32), 2).reshape(128, 1)
    wd14 = np.tile(np.concatenate([w_dense, bias[:1]]).astype(np.float32), (128, 1))

    in_maps = []
    for c in range(NCORES):
        sl = slice(c * BC, (c + 1) * BC)
        sf = np.asarray(sparse_feat[sl], dtype=np.int64)  # [512, 26]
        # host-side gather + transpose: embT[d + 64*g, f*256 + (b%256)]
        g = emb_bf[sf]  # [512, 26, 64]
        embt = np.ascontiguousarray(
            g.reshape(2, 256, F, D).transpose(0, 3, 2, 1).reshape(128, F * 256)
        )
        # lin values: [128, (blk f)] with b = blk*128 + p
        lv = lin_f32[sf]  # [512, 26] f32
        linarr = np.ascontiguousarray(
            lv.reshape(4, 128, F).transpose(1, 0, 2).reshape(128, 4 * F)
        )
        df = np.asarray(dense_feat[sl], dtype=np.float32)
        d14 = np.concatenate([df, np.ones((BC, 1), np.float32)], axis=1)
        d14 = d14.reshape(4, 128, 14).transpose(1, 0, 2).reshape(128, 4 * 14)
        d14 = np.concatenate([d14, wd14], axis=1).copy()  # [128, 70]
        in_maps.append(
            {
                "embt": embt,
                "lin": linarr,
                "dense": d14,
                "w1blk": w1blk,
                "b1c": b1c,
                "w2blk": w2blk,
                "pblk": pblk,
            }
        )
    return in_maps


_NC_CACHE = {}


def kernel(**inputs):
    if "nc" not in _NC_CACHE:
        _NC_CACHE["nc"] = build_nc()
    nc = _NC_CACHE["nc"]
    in_maps = prep_inputs(**inputs)
    res = run_bass_kernel_spmd(nc, in_maps, core_ids=list(range(NCORES)))
    outs = [np.asarray(res.results[c]["out"], dtype=np.float32) for c in range(NCORES)]
    return np.concatenate(outs, axis=0)


# revision 23
# speedup vs baseline: 1.3624x; 1.0250x over previous
"""AFM (Attentional Factorization Machine) Trainium2 kernel, 8-core data-parallel.

Reference computation (per batch row b):
  emb        = embed_table[sparse_feat[b]]                      [26, 64]
  linear_out = sum_f lin_table[sparse_feat[b,f]] + dense.w_d + bias
  att_x[p]   = emb[i_p] * emb[j_p]          (325 pairs i<j)     [325, 64]
  h          = relu(att_x @ W1 + b1);  att_w = h @ w2           [325]
  score      = softmax(att_w);  out = linear_out + (sum_p score_p att_x[p]) . p

Strategy: shard batch (4096 -> 512/core).  Host prep does the embedding
gather AND the transpose, shipping a ready-to-use embT per core:
  embT [128 = d + 64*g, 26*256], g = b//256 (two batch-groups packed on
  partitions), col = f*256 + (b%256).  Device just DMAs it (1.7MB,
  split into 7 four-field parts so chunk compute starts as soon as the
  first fields land).
Per core:
  - pairs processed j-major: chunk j covers pairs (i<j) -> att_x chunk via DVE
    tensor_tensor with step-0 broadcast of emb_j over the i-run, bf16.
  - mm1: lhsT = blockdiag(W1,W1) [128,128], stream attx -> h psum [128,1024].
  - relu+bias via ACT (one op per 1024 cols) -> relu_h bf16.
  - att_w: per 128-col tile, matmul(lhsT=relu_h_tile, rhs=[[w2,0],[0,w2]]) ->
    psum [128 b, 2];  q likewise from attx with [[p],[p]].  b-on-partitions.
  - softmax over pairs without max subtraction (logits are tiny); exp+sum
    fused on ACT via accum_out; out = linear + (sum ew*q)/Z.
"""

import os
import sys

sys.path.insert(0, "/opt/trn_rl_repo")

import numpy as np
import ml_dtypes

from concourse import bass, library_config, mybir, tile
from concourse import bass_utils as _bu
from concourse.bass_utils import run_bass_kernel_spmd
from concourse.masks import make_identity

BF16 = mybir.dt.bfloat16
FP8 = mybir.dt.float8e4
F32 = mybir.dt.float32
I32 = mybir.dt.int32

# emb is pre-scaled x16 on host so h/rh (x256) sit in fp8 e4m3's normal
# range; att_w/q are unscaled (x1/256) on psum readout.
EMB_SCALE = 16.0
UNSCALE = 1.0 / (EMB_SCALE * EMB_SCALE)

V = 100000
F = 26
D = 64
BC = 512  # batch per core
NCORES = 8
NPAIR = F * (F - 1) // 2  # 325
NPART = (F + 3) // 4  # 7 four-field parts


def _split_excess_waits(nc, cap=1):
    """This walrus build gives most instruction structs a single embedded
    sync-wait slot; Tile can emit more.  Hoist extra waits onto injected
    same-engine NoOps right before the instruction (engine FIFO makes the
    combined wait-set semantics identical)."""
    skip = {"EventSemaphore", "AllEngineBarrier", "UnconditionalBranch",
            "CompareAndBranch", "BranchHint"}
    for fn in nc.m.functions:
        for blk in fn.blocks:
            new_insts = []
            for inst in blk.instructions:
                if inst.opcode == "Ldweights" and inst.tile_position == (0, 0):
                    # a (0,0)/full tile_position is a no-op but trips walrus's
                    # "not compatible with LDW optimization" check
                    inst.tile_position = None
                    inst.tile_size = None
                si = inst.sync_info
                if (
                    si is not None
                    and si.on_wait
                    and len(si.on_wait) > cap
                    and inst.opcode not in skip
                ):
                    waits = list(si.on_wait)
                    for w in waits[:-cap]:
                        nop = mybir.InstNoOp(
                            name=nc.get_next_instruction_name(),
                            engine=inst.engine,
                            bass_nofuse=True,
                            sync_info=mybir.SyncInfo(on_wait=[w], on_update=[]),
                        )
                        new_insts.append(nop)
                    si.on_wait = waits[-cap:]
                new_insts.append(inst)
            blk.instructions = new_insts
    return nc


def build_nc(split_waits=True):
    nc = bass.Bass()

    embt_d = nc.declare_dram_parameter("embt", [128, F * 256], BF16, isOutput=False)
    lin_d = nc.declare_dram_parameter("lin", [128, 4 * F], F32, isOutput=False)
    dense = nc.declare_dram_parameter("dense", [128, 70], F32, isOutput=False)
    w1blk = nc.declare_dram_parameter("w1blk", [128, 128], FP8, isOutput=False)
    b1c = nc.declare_dram_parameter("b1c", [128, 1], F32, isOutput=False)
    w2blk = nc.declare_dram_parameter("w2blk", [128, 2], FP8, isOutput=False)
    pblk = nc.declare_dram_parameter("pblk", [128, 2], BF16, isOutput=False)
    out_d = nc.declare_dram_parameter("out", [BC], F32, isOutput=True)

    NTILE = 2 * NPAIR  # 650 column-tiles of 128 (pair x bhalf)
    NWCOL = 2 * NTILE  # 1300 psum output cols (x2 groups)

    with tile.TileContext(nc) as tc:
        with (
            tc.tile_pool(name="const", bufs=1) as cpool,
            tc.tile_pool(name="ax", bufs=4) as axpool,
            tc.tile_pool(name="rh", bufs=4) as rhpool,
            tc.tile_pool(name="ps", bufs=2, space="PSUM") as pspool,
        ):
            # ---- persistent sbuf tensors ----
            embt_p = [
                cpool.tile(
                    [128, min(4, F - 4 * pp) * 256], BF16,
                    tag=f"embt{pp}", name=f"embt{pp}",
                )
                for pp in range(NPART)
            ]
            lin_sb = cpool.tile([128, 4 * F], F32, tag="lin")
            densewd_sb = cpool.tile([128, 70], F32, tag="dense")
            dense_sb = densewd_sb[:, 0:56]
            wd_sb = densewd_sb[:, 56:70]
            w1_sb = cpool.tile([128, 128], FP8, tag="w1")
            b1_sb = cpool.tile([128, 1], F32, tag="b1")
            w2_sb = cpool.tile([128, 2], FP8, tag="w2")
            p_sb = cpool.tile([128, 2], BF16, tag="p")
            ident = cpool.tile([128, 128], BF16, tag="ident")
            attw_sb = cpool.tile([128, NWCOL], F32, tag="attw")
            q_sb = cpool.tile([128, NWCOL], F32, tag="q")
            ew_sb = cpool.tile([128, NWCOL], F32, tag="ew")
            prod_sb = cpool.tile([128, NWCOL], F32, tag="prod")
            lsum_sb = cpool.tile([128, 4], F32, tag="lsum")
            dtmp_sb = cpool.tile([128, 4 * 14], F32, tag="dtmp")
            dsum_sb = cpool.tile([128, 4], F32, tag="dsum")
            lino_sb = cpool.tile([128, 4], F32, tag="lino")
            z_sb = cpool.tile([128, 4], F32, tag="z")
            zi_sb = cpool.tile([128, 4], F32, tag="zi")
            s4_sb = cpool.tile([128, 4], F32, tag="s4")
            fm_sb = cpool.tile([128, 4], F32, tag="fm")
            out_sb = cpool.tile([128, 4], F32, tag="outsb")

            # ---- load parameters ----
            # Small params first (their triggers are fast and W1/b1 gate the
            # first matmul/relu); embt triggers spread across the other
            # engine sequencers so the ~600ns DIRECT2D issues parallelize.
            nc.sync.dma_start(out=w1_sb[:], in_=w1blk[:, :])
            nc.sync.dma_start(out=b1_sb[:], in_=b1c[:, :])
            nc.sync.dma_start(out=w2_sb[:], in_=w2blk[:, :])
            nc.sync.dma_start(out=p_sb[:], in_=pblk[:, :])
            # parts 0/1 split by partition halves (two queues each) so the
            # first chunks' inputs land ~2x sooner; small params on sync.
            for pp in (0, 1):
                for (r0, r1), eng in (((0, 64), nc.scalar), ((64, 128), nc.gpsimd)):
                    eng.dma_start(
                        out=embt_p[pp][r0:r1, :],
                        in_=embt_d[r0:r1, pp * 1024 : pp * 1024 + 1024],
                    )
            trig = [nc.scalar, nc.gpsimd]
            for pp in range(2, NPART):
                nf = min(4, F - 4 * pp)
                trig[pp % 2].dma_start(
                    out=embt_p[pp][:],
                    in_=embt_d[:, pp * 1024 : pp * 1024 + nf * 256],
                )
            nc.gpsimd.dma_start(out=lin_sb[:], in_=lin_d[:, :])
            nc.sync.dma_start(out=densewd_sb[:], in_=dense[:, :])
            make_identity(nc, ident[:])

            # ---- chunk j: pairs (i, j) for i < j ----
            state = {
                "tdone": 0, "copied": 0, "ps_w": None, "ps_q": None,
                "gidx": 0, "axidx": 0, "pend": [],
            }

            def emit_mm2s(rh, ax, n0, N):
                """Second-stage matmuls for one group (emitted two groups
                late so PE never waits on the relu of the group it is
                currently streaming).  w and q share one psum bank."""
                for t0 in range(0, N, 128):
                    u = state["tdone"] % 128
                    if u == 0:
                        state["ps_w"] = pspool.tile(
                            [128, 512], F32, tag="w", name=f"psw{state['tdone']}"
                        )
                    # q first: it depends only on ax, not on the relu output
                    nc.tensor.matmul(
                        out=state["ps_w"][:, 256 + 2 * u : 256 + 2 * u + 2],
                        lhsT=ax[:, n0 + t0 : n0 + t0 + 128],
                        rhs=p_sb[:],
                        start=True,
                        stop=True,
                    )
                    nc.tensor.matmul(
                        out=state["ps_w"][:, 2 * u : 2 * u + 2],
                        lhsT=rh[:, t0 : t0 + 128],
                        rhs=w2_sb[:],
                        start=True,
                        stop=True,
                    )
                    state["tdone"] += 1
                    if state["tdone"] % 128 == 0 or state["tdone"] == NTILE:
                        w2c = 2 * ((state["tdone"] - 1) % 128 + 1)
                        c0 = state["copied"]
                        nc.vector.tensor_scalar(
                            out=attw_sb[:, c0 : c0 + w2c],
                            in0=state["ps_w"][:, :w2c],
                            scalar1=UNSCALE,
                            scalar2=None,
                            op0=mybir.AluOpType.mult,
                        )
                        nc.vector.tensor_scalar(
                            out=q_sb[:, c0 : c0 + w2c],
                            in0=state["ps_w"][:, 256 : 256 + w2c],
                            scalar1=UNSCALE,
                            scalar2=None,
                            op0=mybir.AluOpType.mult,
                        )
                        state["copied"] += w2c

            def emit_chunk(j):
                jpp, jo = j // 4, j % 4
                ax = axpool.tile([128, 25 * 256], BF16, tag="ax", name=f"ax{j}")
                # absorb WAR-on-slot (PE readers) into a 1-wait-capable copy
                nc.vector.tensor_copy(out=ax[:, 0:1], in_=ident[:, 0:1])
                in0 = embt_p[jpp][:, jo * 256 : (jo + 1) * 256].rearrange(
                    "p (one w) -> p one w", one=1
                )
                for ib in range(0, j, 4):
                    w = min(4, j - ib)
                    # NOTE: gpsimd elementwise was tried here and abandoned:
                    # concurrent Q7 SBUF traffic slows DVE ops ~3x.
                    axeng = nc.vector
                    axeng.tensor_tensor(
                        out=ax[:, ib * 256 : (ib + w) * 256].rearrange(
                            "p (l w) -> p l w", l=w
                        ),
                        in0=in0.to_broadcast([128, w, 256]),
                        in1=embt_p[ib // 4][:, : w * 256].rearrange(
                            "p (l w) -> p l w", l=w
                        ),
                        op=mybir.AluOpType.mult,
                    )
                ncols = j * 256
                for n0 in range(0, ncols, 1536):
                    N = min(1536, ncols - n0)
                    hp = pspool.tile([128, 1536], F32, tag="h", name=f"hp{j}_{n0}")
                    for m0 in range(0, N, 512):
                        M = min(512, N - m0)
                        nc.tensor.matmul(
                            out=hp[:, m0 : m0 + M],
                            lhsT=w1_sb[:],
                            rhs=ax[:, n0 + m0 : n0 + m0 + M],
                            start=True,
                            stop=True,
                        )
                    rh = rhpool.tile([128, 1536], FP8, tag="rh", name=f"rh{j}_{n0}")
                    # split relu 6:1 between ACT and DVE
                    if state["gidx"] % 7 < 6:
                        nc.scalar.activation(
                            out=rh[:, :N],
                            in_=hp[:, :N],
                            func=mybir.ActivationFunctionType.Relu,
                            bias=b1_sb[:],
                        )
                    else:
                        nc.vector.tensor_scalar(
                            out=rh[:, :N],
                            in0=hp[:, :N],
                            scalar1=b1_sb[:, 0:1],
                            scalar2=0.0,
                            op0=mybir.AluOpType.add,
                            op1=mybir.AluOpType.max,
                        )
                    state["gidx"] += 1
                    state["pend"].append((rh, ax, n0, N))
                    if len(state["pend"]) > 2:
                        emit_mm2s(*state["pend"].pop(0))

            # Order: small chunks first while embT streams in, big chunks
            # mid-kernel, tiny chunks last so the MM2/relu drain is short.
            order = [1, 2, 3, 7, 8, 9, 10, 11] + list(range(25, 11, -1)) + [4, 6, 5]
            assert sorted(order) == list(range(1, F))
            for j in order:
                emit_chunk(j)
            while state["pend"]:
                emit_mm2s(*state["pend"].pop(0))

            # ---- linear tail ----
            nc.vector.reduce_sum(
                out=lsum_sb[:],
                in_=lin_sb[:].rearrange("p (blk f) -> p blk f", blk=4),
                axis=mybir.AxisListType.X,
            )
            for blk in range(4):
                nc.vector.tensor_tensor(
                    out=dtmp_sb[:, blk * 14 : (blk + 1) * 14],
                    in0=dense_sb[:, blk * 14 : (blk + 1) * 14],
                    in1=wd_sb[:, :],
                    op=mybir.AluOpType.mult,
                )
            nc.vector.reduce_sum(
                out=dsum_sb[:],
                in_=dtmp_sb[:].rearrange("p (blk w) -> p blk w", blk=4),
                axis=mybir.AxisListType.X,
            )
            nc.vector.tensor_add(out=lino_sb[:], in0=lsum_sb[:], in1=dsum_sb[:])

            # ---- softmax (no max subtraction; logits are tiny) ----
            attw_v = attw_sb[:].rearrange("p (a four) -> p a four", four=4)
            ew_v = ew_sb[:].rearrange("p (a four) -> p a four", four=4)
            for beta in range(4):
                nc.scalar.activation(
                    out=ew_v[:, :, beta],
                    in_=attw_v[:, :, beta],
                    func=mybir.ActivationFunctionType.Exp,
                    accum_out=z_sb[:, beta : beta + 1],
                )
            nc.vector.reciprocal(out=zi_sb[:], in_=z_sb[:])
            nc.vector.tensor_copy(out=prod_sb[:, 0:1], in_=ew_sb[:, 0:1])
            nc.vector.tensor_tensor(
                out=prod_sb[:], in0=ew_sb[:], in1=q_sb[:], op=mybir.AluOpType.mult
            )
            nc.vector.reduce_sum(
                out=s4_sb[:],
                in_=prod_sb[:].rearrange("p (a four) -> p four a", four=4),
                axis=mybir.AxisListType.X,
            )
            nc.vector.tensor_tensor(
                out=fm_sb[:], in0=s4_sb[:], in1=zi_sb[:], op=mybir.AluOpType.mult
            )
            # out[b], b = blk*128 + part; fm bucket beta = 2*(blk%2) + blk//2
            otrig = [nc.scalar, nc.gpsimd, nc.sync, nc.gpsimd]
            for blk in range(4):
                beta = 2 * (blk % 2) + blk // 2
                nc.vector.tensor_add(
                    out=out_sb[:, blk : blk + 1],
                    in0=lino_sb[:, blk : blk + 1],
                    in1=fm_sb[:, beta : beta + 1],
                )
                otrig[blk].dma_start(
                    out=out_d[blk * 128 : (blk + 1) * 128],
                    in_=out_sb[:, blk : blk + 1],
                )
    if split_waits:
        _split_excess_waits(nc)
    # populate .instr bytes for extended insts — raw Bass skips this Bacc
    # pass; without it walrus fails "ISA wrong length"
    mybir.codegen_inst_isa_subclasses(nc)
    return nc


def prep_inputs(sparse_feat, dense_feat, embed_table, lin_table, w_dense, bias, W1, b1, w2, p):
    """Host-side preprocessing -> per-core in_maps."""
    sparse_feat = np.asarray(sparse_feat)
    dense_feat = np.asarray(dense_feat, dtype=np.float32)
    embed_table = np.asarray(embed_table, dtype=np.float32)
    lin_table = np.asarray(lin_table, dtype=np.float32)
    w_dense = np.asarray(w_dense, dtype=np.float32)
    bias = np.asarray(bias, dtype=np.float32)
    W1 = np.asarray(W1, dtype=np.float32)
    b1 = np.asarray(b1, dtype=np.float32)
    w2 = np.asarray(w2, dtype=np.float32)
    p = np.asarray(p, dtype=np.float32)
    bf = ml_dtypes.bfloat16
    emb_bf = (embed_table * EMB_SCALE).astype(bf)
    lin_f32 = lin_table.astype(np.float32)

    f8 = ml_dtypes.float8_e4m3
    w1blk = np.zeros((128, 128), dtype=f8)
    w1blk[:D, :D] = W1.astype(f8)
    w1blk[D:, D:] = W1.astype(f8)
    w2blk = np.zeros((128, 2), dtype=f8)
    w2blk[:D, 0] = w2.astype(f8)
    w2blk[D:, 1] = w2.astype(f8)
    pblk = np.zeros((128, 2), dtype=bf)
    pblk[:D, 0] = p.astype(bf)
    pblk[D:, 1] = p.astype(bf)
    # h in psum carries EMB_SCALE^2; bias must match
    b1c = np.tile((b1 * EMB_SCALE * EMB_SCALE).astype(np.float32), 2).reshape(128, 1)
    wd14 = np.tile(np.concatenate([w_dense, bias[:1]]).astype(np.float32), (128, 1))

    in_maps = []
    for c in range(NCORES):
        sl = slice(c * BC, (c + 1) * BC)
        sf = np.asarray(sparse_feat[sl], dtype=np.int64)  # [512, 26]
        # host-side gather + transpose: embT[d + 64*g, f*256 + (b%256)]
        g = emb_bf[sf]  # [512, 26, 64]
        embt = np.ascontiguousarray(
            g.reshape(2, 256, F, D).transpose(0, 3, 2, 1).reshape(128, F * 256)
        )
        # lin values: [128, (blk f)] with b = blk*128 + p
        lv = lin_f32[sf]  # [512, 26] f32
        linarr = np.ascontiguousarray(
            lv.reshape(4, 128, F).transpose(1, 0, 2).reshape(128, 4 * F)
        )
        df = np.asarray(dense_feat[sl], dtype=np.float32)
        d14 = np.concatenate([df, np.ones((BC, 1), np.float32)], axis=1)
        d14 = d14.reshape(4, 128, 14).transpose(1, 0, 2).reshape(128, 4 * 14)
        d14 = np.concatenate([d14, wd14], axis=1).copy()  # [128, 70]
        in_maps.append(
            {
                "embt": embt,
                "lin": linarr,
                "dense": d14,
                "w1blk": w1blk,
                "b1c": b1c,
                "w2blk": w2blk,
                "pblk": pblk,
            }
        )
    return in_maps


_NC_CACHE = {}


def kernel(**inputs):
    if "nc" not in _NC_CACHE:
        _NC_CACHE["nc"] = build_nc()
    nc = _NC_CACHE["nc"]
    in_maps = prep_inputs(**inputs)
    res = run_bass_kernel_spmd(nc, in_maps, core_ids=list(range(NCORES)))
    outs = [np.asarray(res.results[c]["out"], dtype=np.float32) for c in range(NCORES)]
    return np.concatenate(outs, axis=0)


# revision 26
# speedup vs baseline: 1.3905x; 1.0206x over previous
"""AFM (Attentional Factorization Machine) Trainium2 kernel, 8-core data-parallel.

Reference computation (per batch row b):
  emb        = embed_table[sparse_feat[b]]                      [26, 64]
  linear_out = sum_f lin_table[sparse_feat[b,f]] + dense.w_d + bias
  att_x[p]   = emb[i_p] * emb[j_p]          (325 pairs i<j)     [325, 64]
  h          = relu(att_x @ W1 + b1);  att_w = h @ w2           [325]
  score      = softmax(att_w);  out = linear_out + (sum_p score_p att_x[p]) . p

Strategy: shard batch (4096 -> 512/core).  Host prep does the embedding
gather AND the transpose, shipping a ready-to-use embT per core:
  embT [128 = d + 64*g, 26*256], g = b//256 (two batch-groups packed on
  partitions), col = f*256 + (b%256).  Device just DMAs it (1.7MB,
  split into 7 four-field parts so chunk compute starts as soon as the
  first fields land).
Per core:
  - pairs processed j-major: chunk j covers pairs (i<j) -> att_x chunk via DVE
    tensor_tensor with step-0 broadcast of emb_j over the i-run, bf16.
  - mm1: lhsT = blockdiag(W1,W1) [128,128], stream attx -> h psum [128,1024].
  - relu+bias via ACT (one op per 1024 cols) -> relu_h bf16.
  - att_w: per 128-col tile, matmul(lhsT=relu_h_tile, rhs=[[w2,0],[0,w2]]) ->
    psum [128 b, 2];  q likewise from attx with [[p],[p]].  b-on-partitions.
  - softmax over pairs without max subtraction (logits are tiny); exp+sum
    fused on ACT via accum_out; out = linear + (sum ew*q)/Z.
"""

import os
import sys

sys.path.insert(0, "/opt/trn_rl_repo")

import numpy as np
import ml_dtypes

from concourse import bass, library_config, mybir, tile
from concourse import bass_utils as _bu
from concourse.bass_utils import run_bass_kernel_spmd
from concourse.masks import make_identity

BF16 = mybir.dt.bfloat16
FP8 = mybir.dt.float8e4
F32 = mybir.dt.float32
I32 = mybir.dt.int32

# emb is pre-scaled x16 on host so h/rh (x256) sit in fp8 e4m3's normal
# range; att_w/q are unscaled (x1/256) on psum readout.
EMB_SCALE = 16.0
UNSCALE = 1.0 / (EMB_SCALE * EMB_SCALE)

V = 100000
F = 26
D = 64
BC = 512  # batch per core
NCORES = 8
NPAIR = F * (F - 1) // 2  # 325
NPART = (F + 3) // 4  # 7 four-field parts


def _split_excess_waits(nc, cap=1):
    """This walrus build gives most instruction structs a single embedded
    sync-wait slot; Tile can emit more.  Hoist extra waits onto injected
    same-engine NoOps right before the instruction (engine FIFO makes the
    combined wait-set semantics identical)."""
    skip = {"EventSemaphore", "AllEngineBarrier", "UnconditionalBranch",
            "CompareAndBranch", "BranchHint"}
    for fn in nc.m.functions:
        for blk in fn.blocks:
            new_insts = []
            for inst in blk.instructions:
                if inst.opcode == "Ldweights" and inst.tile_position == (0, 0):
                    # a (0,0)/full tile_position is a no-op but trips walrus's
                    # "not compatible with LDW optimization" check
                    inst.tile_position = None
                    inst.tile_size = None
                si = inst.sync_info
                if (
                    si is not None
                    and si.on_wait
                    and len(si.on_wait) > cap
                    and inst.opcode not in skip
                ):
                    waits = list(si.on_wait)
                    for w in waits[:-cap]:
                        nop = mybir.InstNoOp(
                            name=nc.get_next_instruction_name(),
                            engine=inst.engine,
                            bass_nofuse=True,
                            sync_info=mybir.SyncInfo(on_wait=[w], on_update=[]),
                        )
                        new_insts.append(nop)
                    si.on_wait = waits[-cap:]
                new_insts.append(inst)
            blk.instructions = new_insts
    return nc


def build_nc(split_waits=True):
    nc = bass.Bass()

    embt_d = nc.declare_dram_parameter("embt", [128, F * 256], BF16, isOutput=False)
    lin_d = nc.declare_dram_parameter("lin", [128, 4 * F], F32, isOutput=False)
    dense = nc.declare_dram_parameter("dense", [128, 70], F32, isOutput=False)
    w1blk = nc.declare_dram_parameter("w1blk", [128, 128], FP8, isOutput=False)
    b1c = nc.declare_dram_parameter("b1c", [128, 1], F32, isOutput=False)
    w2blk = nc.declare_dram_parameter("w2blk", [128, 2], FP8, isOutput=False)
    pblk = nc.declare_dram_parameter("pblk", [128, 2], BF16, isOutput=False)
    out_d = nc.declare_dram_parameter("out", [BC], F32, isOutput=True)

    NTILE = 2 * NPAIR  # 650 column-tiles of 128 (pair x bhalf)
    NWCOL = 2 * NTILE  # 1300 psum output cols (x2 groups)

    with tile.TileContext(nc) as tc:
        with (
            tc.tile_pool(name="const", bufs=1) as cpool,
            tc.tile_pool(name="ax", bufs=4) as axpool,
            tc.tile_pool(name="rh", bufs=4) as rhpool,
            tc.tile_pool(name="ps", bufs=2, space="PSUM") as pspool,
        ):
            # ---- persistent sbuf tensors ----
            embt_p = [
                cpool.tile(
                    [128, min(4, F - 4 * pp) * 256], BF16,
                    tag=f"embt{pp}", name=f"embt{pp}",
                )
                for pp in range(NPART)
            ]
            lin_sb = cpool.tile([128, 4 * F], F32, tag="lin")
            densewd_sb = cpool.tile([128, 70], F32, tag="dense")
            dense_sb = densewd_sb[:, 0:56]
            wd_sb = densewd_sb[:, 56:70]
            w1_sb = cpool.tile([128, 128], FP8, tag="w1")
            b1_sb = cpool.tile([128, 1], F32, tag="b1")
            w2_sb = cpool.tile([128, 2], FP8, tag="w2")
            p_sb = cpool.tile([128, 2], BF16, tag="p")
            ident = cpool.tile([128, 128], BF16, tag="ident")
            attw_sb = cpool.tile([128, NWCOL], F32, tag="attw")
            q_sb = cpool.tile([128, NWCOL], F32, tag="q")
            ew_sb = cpool.tile([128, NWCOL], F32, tag="ew")
            prod_sb = cpool.tile([128, NWCOL], F32, tag="prod")
            lsum_sb = cpool.tile([128, 4], F32, tag="lsum")
            dtmp_sb = cpool.tile([128, 4 * 14], F32, tag="dtmp")
            dsum_sb = cpool.tile([128, 4], F32, tag="dsum")
            lino_sb = cpool.tile([128, 4], F32, tag="lino")
            z_sb = cpool.tile([128, 4], F32, tag="z")
            zi_sb = cpool.tile([128, 4], F32, tag="zi")
            s4_sb = cpool.tile([128, 4], F32, tag="s4")
            fm_sb = cpool.tile([128, 4], F32, tag="fm")
            out_sb = cpool.tile([128, 4], F32, tag="outsb")

            # ---- load parameters ----
            # Small params first (their triggers are fast and W1/b1 gate the
            # first matmul/relu); embt triggers spread across the other
            # engine sequencers so the ~600ns DIRECT2D issues parallelize.
            nc.sync.dma_start(out=w1_sb[:], in_=w1blk[:, :])
            nc.sync.dma_start(out=b1_sb[:], in_=b1c[:, :])
            nc.sync.dma_start(out=w2_sb[:], in_=w2blk[:, :])
            nc.sync.dma_start(out=p_sb[:], in_=pblk[:, :])
            # parts 0/1 split by partition halves (two queues each) so the
            # first chunks' inputs land ~2x sooner; small params on sync.
            for pp in (0, 1):
                for (r0, r1), eng in (((0, 64), nc.scalar), ((64, 128), nc.gpsimd)):
                    eng.dma_start(
                        out=embt_p[pp][r0:r1, :],
                        in_=embt_d[r0:r1, pp * 1024 : pp * 1024 + 1024],
                    )
            trig = [nc.scalar, nc.gpsimd]
            for pp in range(2, NPART):
                nf = min(4, F - 4 * pp)
                trig[pp % 2].dma_start(
                    out=embt_p[pp][:],
                    in_=embt_d[:, pp * 1024 : pp * 1024 + nf * 256],
                )
            nc.gpsimd.dma_start(out=lin_sb[:], in_=lin_d[:, :])
            nc.sync.dma_start(out=densewd_sb[:], in_=dense[:, :])
            make_identity(nc, ident[:])

            # ---- chunk j: pairs (i, j) for i < j ----
            state = {
                "tdone": 0, "copied": 0, "ps_w": None, "ps_q": None,
                "gidx": 0, "axidx": 0, "pend": [],
            }

            def emit_mm2s(rh, ax, n0, N):
                """Second-stage matmuls for one group (emitted two groups
                late so PE never waits on the relu of the group it is
                currently streaming).  w and q share one psum bank."""
                for t0 in range(0, N, 128):
                    u = state["tdone"] % 128
                    if u == 0:
                        state["ps_w"] = pspool.tile(
                            [128, 512], F32, tag="w", name=f"psw{state['tdone']}"
                        )
                    # q first: it depends only on ax, not on the relu output
                    nc.tensor.matmul(
                        out=state["ps_w"][:, 256 + 2 * u : 256 + 2 * u + 2],
                        lhsT=ax[:, n0 + t0 : n0 + t0 + 128],
                        rhs=p_sb[:],
                        start=True,
                        stop=True,
                    )
                    nc.tensor.matmul(
                        out=state["ps_w"][:, 2 * u : 2 * u + 2],
                        lhsT=rh[:, t0 : t0 + 128],
                        rhs=w2_sb[:],
                        start=True,
                        stop=True,
                    )
                    state["tdone"] += 1
                    if state["tdone"] % 128 == 0 or state["tdone"] == NTILE:
                        w2c = 2 * ((state["tdone"] - 1) % 128 + 1)
                        c0 = state["copied"]
                        nc.vector.tensor_scalar(
                            out=attw_sb[:, c0 : c0 + w2c],
                            in0=state["ps_w"][:, :w2c],
                            scalar1=UNSCALE,
                            scalar2=None,
                            op0=mybir.AluOpType.mult,
                        )
                        nc.vector.tensor_scalar(
                            out=q_sb[:, c0 : c0 + w2c],
                            in0=state["ps_w"][:, 256 : 256 + w2c],
                            scalar1=UNSCALE,
                            scalar2=None,
                            op0=mybir.AluOpType.mult,
                        )
                        state["copied"] += w2c

            def emit_chunk(j):
                jpp, jo = j // 4, j % 4
                ax = axpool.tile([128, 25 * 256], BF16, tag="ax", name=f"ax{j}")
                # absorb WAR-on-slot (PE readers) into a 1-wait-capable copy
                nc.vector.tensor_copy(out=ax[:, 0:1], in_=ident[:, 0:1])
                in0 = embt_p[jpp][:, jo * 256 : (jo + 1) * 256].rearrange(
                    "p (one w) -> p one w", one=1
                )
                for ib in range(0, j, 4):
                    w = min(4, j - ib)
                    # NOTE: gpsimd elementwise was tried here and abandoned:
                    # concurrent Q7 SBUF traffic slows DVE ops ~3x.
                    axeng = nc.vector
                    axeng.tensor_tensor(
                        out=ax[:, ib * 256 : (ib + w) * 256].rearrange(
                            "p (l w) -> p l w", l=w
                        ),
                        in0=embt_p[ib // 4][:, : w * 256].rearrange(
                            "p (l w) -> p l w", l=w
                        ),
                        in1=in0.to_broadcast([128, w, 256]),
                        op=mybir.AluOpType.mult,
                    )
                ncols = j * 256
                for n0 in range(0, ncols, 1536):
                    N = min(1536, ncols - n0)
                    hp = pspool.tile([128, 1536], F32, tag="h", name=f"hp{j}_{n0}")
                    for m0 in range(0, N, 512):
                        M = min(512, N - m0)
                        nc.tensor.matmul(
                            out=hp[:, m0 : m0 + M],
                            lhsT=w1_sb[:],
                            rhs=ax[:, n0 + m0 : n0 + m0 + M],
                            start=True,
                            stop=True,
                        )
                    rh = rhpool.tile([128, 1536], FP8, tag="rh", name=f"rh{j}_{n0}")
                    # split relu 6:1 between ACT and DVE; alternate 1:1 over
                    # the final groups so the end-of-kernel drain is parallel
                    if state["gidx"] >= TOT_GROUPS - 8:
                        use_act = state["gidx"] % 2 == 0
                    else:
                        use_act = state["gidx"] % 7 < 6
                    if use_act:
                        nc.scalar.activation(
                            out=rh[:, :N],
                            in_=hp[:, :N],
                            func=mybir.ActivationFunctionType.Relu,
                            bias=b1_sb[:],
                        )
                    else:
                        nc.vector.tensor_scalar(
                            out=rh[:, :N],
                            in0=hp[:, :N],
                            scalar1=b1_sb[:, 0:1],
                            scalar2=0.0,
                            op0=mybir.AluOpType.add,
                            op1=mybir.AluOpType.max,
                        )
                    state["gidx"] += 1
                    state["pend"].append((rh, ax, n0, N))
                    if len(state["pend"]) > 2:
                        emit_mm2s(*state["pend"].pop(0))

            # Order: small chunks first while embT streams in, big chunks
            # mid-kernel, tiny chunks last so the MM2/relu drain is short.
            order = [1, 2, 3, 7, 8, 9, 10, 11] + list(range(25, 11, -1)) + [4, 6, 5]
            assert sorted(order) == list(range(1, F))
            TOT_GROUPS = sum(-(-(j * 256) // 1536) for j in order)
            for j in order:
                emit_chunk(j)
            while state["pend"]:
                emit_mm2s(*state["pend"].pop(0))

            # ---- linear tail ----
            nc.vector.reduce_sum(
                out=lsum_sb[:],
                in_=lin_sb[:].rearrange("p (blk f) -> p blk f", blk=4),
                axis=mybir.AxisListType.X,
            )
            for blk in range(4):
                nc.vector.tensor_tensor(
                    out=dtmp_sb[:, blk * 14 : (blk + 1) * 14],
                    in0=dense_sb[:, blk * 14 : (blk + 1) * 14],
                    in1=wd_sb[:, :],
                    op=mybir.AluOpType.mult,
                )
            nc.vector.reduce_sum(
                out=dsum_sb[:],
                in_=dtmp_sb[:].rearrange("p (blk w) -> p blk w", blk=4),
                axis=mybir.AxisListType.X,
            )
            nc.vector.tensor_add(out=lino_sb[:], in0=lsum_sb[:], in1=dsum_sb[:])

            # ---- softmax (no max subtraction; logits are tiny) ----
            attw_v = attw_sb[:].rearrange("p (a four) -> p a four", four=4)
            ew_v = ew_sb[:].rearrange("p (a four) -> p a four", four=4)
            for beta in range(4):
                nc.scalar.activation(
                    out=ew_v[:, :, beta],
                    in_=attw_v[:, :, beta],
                    func=mybir.ActivationFunctionType.Exp,
                    accum_out=z_sb[:, beta : beta + 1],
                )
            nc.vector.reciprocal(out=zi_sb[:], in_=z_sb[:])
            nc.vector.tensor_copy(out=prod_sb[:, 0:1], in_=ew_sb[:, 0:1])
            nc.vector.tensor_tensor(
                out=prod_sb[:], in0=ew_sb[:], in1=q_sb[:], op=mybir.AluOpType.mult
            )
            nc.vector.reduce_sum(
                out=s4_sb[:],
                in_=prod_sb[:].rearrange("p (a four) -> p four a", four=4),
                axis=mybir.AxisListType.X,
            )
            nc.vector.tensor_tensor(
                out=fm_sb[:], in0=s4_sb[:], in1=zi_sb[:], op=mybir.AluOpType.mult
            )
            # out[b], b = blk*128 + part; fm bucket beta = 2*(blk%2) + blk//2
            otrig = [nc.scalar, nc.gpsimd, nc.sync, nc.gpsimd]
            for blk in range(4):
                beta = 2 * (blk % 2) + blk // 2
                nc.vector.tensor_add(
                    out=out_sb[:, blk : blk + 1],
                    in0=lino_sb[:, blk : blk + 1],
                    in1=fm_sb[:, beta : beta + 1],
                )
                otrig[blk].dma_start(
                    out=out_d[blk * 128 : (blk + 1) * 128],
                    in_=out_sb[:, blk : blk + 1],
                )
    if split_waits:
        _split_excess_waits(nc)
    # populate .instr bytes for extended insts — raw Bass skips this Bacc
    # pass; without it walrus fails "ISA wrong length"
    mybir.codegen_inst_isa_subclasses(nc)
    return nc


def prep_inputs(sparse_feat, dense_feat, embed_table, lin_table, w_dense, bias, W1, b1, w2, p):
    """Host-side preprocessing -> per-core in_maps."""
    sparse_feat = np.asarray(sparse_feat)
    dense_feat = np.asarray(dense_feat, dtype=np.float32)
    embed_table = np.asarray(embed_table, dtype=np.float32)
    lin_table = np.asarray(lin_table, dtype=np.float32)
    w_dense = np.asarray(w_dense, dtype=np.float32)
    bias = np.asarray(bias, dtype=np.float32)
    W1 = np.asarray(W1, dtype=np.float32)
    b1 = np.asarray(b1, dtype=np.float32)
    w2 = np.asarray(w2, dtype=np.float32)
    p = np.asarray(p, dtype=np.float32)
    bf = ml_dtypes.bfloat16
    emb_bf = (embed_table * EMB_SCALE).astype(bf)
    lin_f32 = lin_table.astype(np.float32)

    f8 = ml_dtypes.float8_e4m3
    w1blk = np.zeros((128, 128), dtype=f8)
    w1blk[:D, :D] = W1.astype(f8)
    w1blk[D:, D:] = W1.astype(f8)
    w2blk = np.zeros((128, 2), dtype=f8)
    w2blk[:D, 0] = w2.astype(f8)
    w2blk[D:, 1] = w2.astype(f8)
    pblk = np.zeros((128, 2), dtype=bf)
    pblk[:D, 0] = p.astype(bf)
    pblk[D:, 1] = p.astype(bf)
    # h in psum carries EMB_SCALE^2; bias must match
    b1c = np.tile((b1 * EMB_SCALE * EMB_SCALE).astype(np.float32), 2).reshape(128, 1)
    wd14 = np.tile(np.concatenate([w_dense, bias[:1]]).astype(np.float32), (128, 1))

    in_maps = []
    for c in range(NCORES):
        sl = slice(c * BC, (c + 1) * BC)
        sf = np.asarray(sparse_feat[sl], dtype=np.int64)  # [512, 26]
        # host-side gather + transpose: embT[d + 64*g, f*256 + (b%256)]
        g = emb_bf[sf]  # [512, 26, 64]
        embt = np.ascontiguousarray(
            g.reshape(2, 256, F, D).transpose(0, 3, 2, 1).reshape(128, F * 256)
        )
        # lin values: [128, (blk f)] with b = blk*128 + p
        lv = lin_f32[sf]  # [512, 26] f32
        linarr = np.ascontiguousarray(
            lv.reshape(4, 128, F).transpose(1, 0, 2).reshape(128, 4 * F)
        )
        df = np.asarray(dense_feat[sl], dtype=np.float32)
        d14 = np.concatenate([df, np.ones((BC, 1), np.float32)], axis=1)
        d14 = d14.reshape(4, 128, 14).transpose(1, 0, 2).reshape(128, 4 * 14)
        d14 = np.concatenate([d14, wd14], axis=1).copy()  # [128, 70]
        in_maps.append(
            {
                "embt": embt,
                "lin": linarr,
                "dense": d14,
                "w1blk": w1blk,
                "b1c": b1c,
                "w2blk": w2blk,
                "pblk": pblk,
            }
        )
    return in_maps


_NC_CACHE = {}


def kernel(**inputs):
    if "nc" not in _NC_CACHE:
        _NC_CACHE["nc"] = build_nc()
    nc = _NC_CACHE["nc"]
    in_maps = prep_inputs(**inputs)
    res = run_bass_kernel_spmd(nc, in_maps, core_ids=list(range(NCORES)))
    outs = [np.asarray(res.results[c]["out"], dtype=np.float32) for c in range(NCORES)]
    return np.concatenate(outs, axis=0)
